# revision 8
# baseline (speedup 1.0000x reference)
"""Dense transformer block (QKV -> causal attention -> out-proj -> FFN+ReLU)
on 8 Trainium2 NeuronCores, data-parallel over the batch dimension.

Contract: kernel(**inputs) takes the FULL inputs
  x [8, 1024, 1024] f32, Wq/Wk/Wv/Wo/W1 [1024, 1024] f32, bo/b1 [1024] f32
and returns the FULL output [8, 1024, 1024] f32.

Each of the 8 cores runs the identical single-core program on one batch
element (batch=8, cores=8 -> no collectives needed).

Single-core design (bf16 tensor-engine compute, fp32 accumulation):
  - x is DMA'd with an fp32->bf16 casting DMA (gpsimd SWDGE), then
    DMA-transposed (xbar) into feature-major xT [E, T].
  - qT, kT are produced feature-major (lhsT=W, rhs=xT); v is produced
    token-major with an extra all-ones column per head ("augmented V").
  - scores are computed TRANSPOSED, sT[t2, t1] = kT_h^T-style matmul, so
    softmax normalization sums arrive for free as the augmented-V row of
    the attn@v matmul (row Dh holds sum_t2 exp(s)).
  - causality: score blocks entirely in the future are skipped, partially
    valid blocks only compute their valid column range, and the single
    diagonal 128x128 sub-block is masked with affine_select after exp.
  - attention output is accumulated feature-major (= exactly the lhsT the
    output projection needs); out-proj emits projT feature-major (= the
    lhsT the FFN needs); FFN emits token-major, bias folded in via a K=1
    matmul of an ones-row with the b1 row, ReLU on PSUM eviction.
"""

import numpy as np
from contextlib import ExitStack

import concourse.bass as bass
import concourse.bacc as bacc
import concourse.tile as tile
from concourse import mybir
from concourse.bass_utils import run_bass_kernel_spmd

F32 = mybir.dt.float32
BF16 = mybir.dt.bfloat16

N_CORES = 8
BATCH = 8
T = 1024
E = 1024
H = 16
DH = 64


def build_nc(TT=T, EE=E, HH=H, Dh=DH):
    nc = bacc.Bacc("TRN2", target_bir_lowering=False)

    x = nc.dram_tensor("x", [TT, EE], F32, kind="ExternalInput")
    Wq = nc.dram_tensor("Wq", [EE, EE], F32, kind="ExternalInput")
    Wk = nc.dram_tensor("Wk", [EE, EE], F32, kind="ExternalInput")
    Wv = nc.dram_tensor("Wv", [EE, EE], F32, kind="ExternalInput")
    Wo = nc.dram_tensor("Wo", [EE, EE], F32, kind="ExternalInput")
    bo = nc.dram_tensor("bo", [EE], F32, kind="ExternalInput")
    W1 = nc.dram_tensor("W1", [EE, EE], F32, kind="ExternalInput")
    b1 = nc.dram_tensor("b1", [EE], F32, kind="ExternalInput")
    out = nc.dram_tensor("out", [TT, EE], F32, kind="ExternalOutput")

    EC = EE // 128          # feature-chunk count (partition tiles)
    TC = TT // 128          # token-chunk count
    QT = min(512, TT)       # t1 (query) free-dim chunk
    NT = TT // QT
    QE = min(512, EE)       # output-feature free-dim chunk
    NE = EE // QE
    HP = 128 // Dh          # heads per 128-partition feature tile
    scale = float(Dh) ** -0.5
    Exp = mybir.ActivationFunctionType.Exp
    Relu = mybir.ActivationFunctionType.Relu
    Ident = mybir.ActivationFunctionType.Identity

    with ExitStack() as ctx:
        tc = ctx.enter_context(tile.TileContext(nc))
        wpool = ctx.enter_context(tc.tile_pool(name="w", bufs=EC + 2))
        xtokp = ctx.enter_context(tc.tile_pool(name="xtok", bufs=2))
        xTp = ctx.enter_context(tc.tile_pool(name="xT", bufs=EC))
        qTp = ctx.enter_context(tc.tile_pool(name="qT", bufs=EC))
        kTp = ctx.enter_context(tc.tile_pool(name="kT", bufs=EC))
        vp = ctx.enter_context(tc.tile_pool(name="v", bufs=TC))
        pp = ctx.enter_context(tc.tile_pool(name="p", bufs=10))
        rtp = ctx.enter_context(tc.tile_pool(name="rt", bufs=4))
        rbp = ctx.enter_context(tc.tile_pool(name="rb", bufs=4))
        aoutp = ctx.enter_context(tc.tile_pool(name="aout", bufs=EC))
        projp = ctx.enter_context(tc.tile_pool(name="proj", bufs=EC))
        constp = ctx.enter_context(tc.tile_pool(name="const", bufs=1))
        ffoutp = ctx.enter_context(tc.tile_pool(name="ffout", bufs=3))
        ps_acc = ctx.enter_context(tc.tile_pool(name="ps_acc", bufs=3, space="PSUM"))
        ps_s = ctx.enter_context(tc.tile_pool(name="ps_s", bufs=3, space="PSUM"))
        ps_o = ctx.enter_context(tc.tile_pool(name="ps_o", bufs=2, space="PSUM"))

        # ---- constants ----
        bo_sb = constp.tile([128, EC], F32)
        nc.sync.dma_start(out=bo_sb, in_=bo.rearrange("(c p) -> p c", p=128))
        b1_sb = constp.tile([1, EE], BF16)
        nc.gpsimd.dma_start(out=b1_sb, in_=b1.rearrange("(a e) -> a e", a=1))
        ones_t = constp.tile([1, 128], BF16)
        nc.vector.memset(ones_t, 1.0)
        ident = constp.tile([128, 128], BF16)
        from concourse.masks import make_identity
        make_identity(nc, ident)

        # ---- x: cast to bf16, transpose to feature-major xT [E, T] ----
        xT = [xTp.tile([128, TT], BF16, name="xT", tag="xT") for _ in range(EC)]
        for ti in range(TC):
            xtok = xtokp.tile([128, EE], BF16)
            nc.gpsimd.dma_start(out=xtok, in_=x[128 * ti:128 * (ti + 1), :])
            for ec in range(EC):
                ps_t = ps_acc.tile([128, 128], BF16, name="ps_t", tag="ps_acc")
                nc.tensor.transpose(
                    ps_t, xtok[:, 128 * ec:128 * (ec + 1)], ident
                )
                nc.vector.tensor_copy(
                    out=xT[ec][:, 128 * ti:128 * (ti + 1)], in_=ps_t
                )

        def load_w(wdram):
            tiles = []
            for ei in range(EC):
                wt = wpool.tile([128, EE], BF16, tag="w")
                nc.gpsimd.dma_start(out=wt, in_=wdram[128 * ei:128 * (ei + 1), :])
                tiles.append(wt)
            return tiles

        # ---- feature-major projection: returns EC tiles [128, T] ----
        def proj_feature_major(wtiles, dstpool):
            outs = [dstpool.tile([128, TT], BF16, name="pfm", tag="pfm") for _ in range(EC)]
            for eo in range(EC):
                pss = [ps_acc.tile([128, QT], F32, name="ps_acc", tag="ps_acc") for _ in range(NT)]
                for ei in range(EC):
                    for t1 in range(NT):
                        nc.tensor.matmul(
                            pss[t1],
                            lhsT=wtiles[ei][:, 128 * eo:128 * (eo + 1)],
                            rhs=xT[ei][:, QT * t1:QT * (t1 + 1)],
                            start=(ei == 0),
                            stop=(ei == EC - 1),
                        )
                for t1 in range(NT):
                    nc.vector.tensor_copy(
                        out=outs[eo][:, QT * t1:QT * (t1 + 1)], in_=pss[t1]
                    )
            return outs

        wq = load_w(Wq)
        qT = proj_feature_major(wq, qTp)
        wk = load_w(Wk)
        kT = proj_feature_major(wk, kTp)

        # ---- v: token-major augmented [128, H*(Dh+1)] per token chunk ----
        wv = load_w(Wv)
        vaug = []
        for ti in range(TC):
            va = vp.tile([128, HH * (Dh + 1)], BF16)
            nc.gpsimd.memset(va, 1.0)
            for eoq in range(NE):
                ps = ps_acc.tile([128, QE], F32, name="ps_acc", tag="ps_acc")
                for ei in range(EC):
                    nc.tensor.matmul(
                        ps,
                        lhsT=xT[ei][:, 128 * ti:128 * (ti + 1)],
                        rhs=wv[ei][:, QE * eoq:QE * (eoq + 1)],
                        start=(ei == 0),
                        stop=(ei == EC - 1),
                    )
                hq = QE // Dh  # heads covered by this chunk
                dst = va[:, (Dh + 1) * hq * eoq:(Dh + 1) * hq * (eoq + 1)]
                dst = dst.rearrange("p (h c) -> p h c", c=Dh + 1)[:, :, 0:Dh]
                src = ps.rearrange("p (h d) -> p h d", d=Dh)
                nc.scalar.copy(out=dst, in_=src)
            vaug.append(va)

        # ---- attention ----
        aoutT = [aoutp.tile([128, TT], BF16, name="aoutT", tag="aoutT") for _ in range(EC)]
        for h in range(HH):
            ecq = h // HP
            po = (h % HP) * Dh
            kTh = kT[ecq][po:po + Dh, :]
            qTh = qT[ecq][po:po + Dh, :]
            for t1 in range(NT):
                t2cs = [t2 for t2 in range(TC) if 128 * t2 < QT * (t1 + 1)]
                ops = ps_o.tile([Dh + 1, QT], F32)
                pts = []
                for t2 in t2cs:
                    k0 = 128 * t2 - QT * t1
                    c0 = max(0, k0)
                    sp = ps_s.tile([128, QT], F32)
                    nc.tensor.matmul(
                        sp[:, c0:QT],
                        lhsT=kTh[:, 128 * t2:128 * (t2 + 1)],
                        rhs=qTh[:, QT * t1 + c0:QT * (t1 + 1)],
                        start=True,
                        stop=True,
                    )
                    pt = pp.tile([128, QT], BF16)
                    nc.scalar.activation(
                        out=pt[:, c0:QT], in_=sp[:, c0:QT], func=Exp, scale=scale
                    )
                    if k0 >= 0:
                        # diagonal 128x128 sub-block: zero strictly-future pairs
                        nc.gpsimd.affine_select(
                            out=pt[:, c0:c0 + 128],
                            in_=pt[:, c0:c0 + 128],
                            compare_op=mybir.AluOpType.is_ge,
                            fill=0.0,
                            base=0,
                            pattern=[[1, 128]],
                            channel_multiplier=-1,
                        )
                    pts.append((pt, c0))
                for j, t2 in enumerate(t2cs):
                    pt, c0 = pts[j]
                    va_h = vaug[t2][:, h * (Dh + 1):(h + 1) * (Dh + 1)]
                    nc.tensor.matmul(
                        ops[:, c0:QT],
                        lhsT=va_h,
                        rhs=pt[:, c0:QT],
                        start=(j == 0),
                        stop=(j == len(t2cs) - 1),
                    )
                rt = rtp.tile([1, QT], F32)
                nc.vector.reciprocal(out=rt, in_=ops[Dh:Dh + 1, :])
                rb = rbp.tile([Dh, QT], F32)
                rt_bcast = bass.AP(
                    tensor=rt.tensor,
                    offset=rt.offset,
                    ap=[[1, 1], [0, Dh]] + rt.ap[1:],
                )
                nc.gpsimd.dma_start(out=rb, in_=rt_bcast)
                nc.vector.tensor_mul(
                    out=aoutT[ecq][po:po + Dh, QT * t1:QT * (t1 + 1)],
                    in0=ops[0:Dh, :],
                    in1=rb,
                )

        # ---- output projection: projT feature-major, bias bo fused ----
        wo = load_w(Wo)
        projT = [projp.tile([128, TT], BF16, name="projT", tag="projT") for _ in range(EC)]
        for eo in range(EC):
            pss = [ps_acc.tile([128, QT], F32, name="ps_acc", tag="ps_acc") for _ in range(NT)]
            for ei in range(EC):
                for t1 in range(NT):
                    nc.tensor.matmul(
                        pss[t1],
                        lhsT=wo[ei][:, 128 * eo:128 * (eo + 1)],
                        rhs=aoutT[ei][:, QT * t1:QT * (t1 + 1)],
                        start=(ei == 0),
                        stop=(ei == EC - 1),
                    )
            for t1 in range(NT):
                nc.scalar.activation(
                    out=projT[eo][:, QT * t1:QT * (t1 + 1)],
                    in_=pss[t1],
                    func=Ident,
                    bias=bo_sb[:, eo:eo + 1],
                    scale=1.0,
                )

        # ---- FFN: relu(proj @ W1 + b1), token-major, streamed to DRAM ----
        w1 = load_w(W1)
        for ti in range(TC):
            for eoq in range(NE):
                ps = ps_acc.tile([128, QE], F32, name="ps_acc", tag="ps_acc")
                for ei in range(EC):
                    nc.tensor.matmul(
                        ps,
                        lhsT=projT[ei][:, 128 * ti:128 * (ti + 1)],
                        rhs=w1[ei][:, QE * eoq:QE * (eoq + 1)],
                        start=(ei == 0),
                        stop=False,
                    )
                nc.tensor.matmul(
                    ps,
                    lhsT=ones_t[:, 0:128],
                    rhs=b1_sb[:, QE * eoq:QE * (eoq + 1)],
                    start=False,
                    stop=True,
                )
                fo = ffoutp.tile([128, QE], F32)
                nc.scalar.activation(out=fo, in_=ps, func=Relu)
                nc.sync.dma_start(
                    out=out[128 * ti:128 * (ti + 1), QE * eoq:QE * (eoq + 1)],
                    in_=fo,
                )

    nc.finalize()
    return nc


_NC_CACHE = {}


def _get_nc(shape_key):
    if shape_key not in _NC_CACHE:
        _NC_CACHE[shape_key] = build_nc(*shape_key)
    return _NC_CACHE[shape_key]


def kernel(x, Wq, Wk, Wv, Wo, bo, W1, b1):
    x = np.ascontiguousarray(np.asarray(x, dtype=np.float32))
    ws = {
        "Wq": np.ascontiguousarray(np.asarray(Wq, dtype=np.float32)),
        "Wk": np.ascontiguousarray(np.asarray(Wk, dtype=np.float32)),
        "Wv": np.ascontiguousarray(np.asarray(Wv, dtype=np.float32)),
        "Wo": np.ascontiguousarray(np.asarray(Wo, dtype=np.float32)),
        "bo": np.ascontiguousarray(np.asarray(bo, dtype=np.float32)),
        "W1": np.ascontiguousarray(np.asarray(W1, dtype=np.float32)),
        "b1": np.ascontiguousarray(np.asarray(b1, dtype=np.float32)),
    }
    B, TT, EE = x.shape
    assert B == N_CORES
    nc = _get_nc((TT, EE, H, DH))
    in_maps = [dict(ws, x=x[b]) for b in range(B)]
    res = run_bass_kernel_spmd(nc, in_maps, core_ids=list(range(N_CORES)))
    return np.stack([res.results[b]["out"] for b in range(B)], axis=0).astype(
        np.float32
    )
